# revision 15
# baseline (speedup 1.0000x reference)
"""BlockConv2D Trainium2 kernel.

Reference computation (see harness): gather 16 blocks of 32 input channels
(indices blocks_in) from x[16,64,64,512], run a per-block 3x3 'same' conv
(weights [16,3,3,32,32]), scatter-add the 16x32 output channels back to 512
channels (indices blocks_out), add bias, relu.

Mapping: groups are independent 32->32 channel convs. Four groups' 32x32
weight blocks pack into one 128x128 block-diagonal stationary operand, so
each tap of the conv is a single matmul per 128-channel tile:
    psum[co_tile, spatial] += W[tap, ctile].T @ x[ctile, spatial+shift]
accumulated over the 9 taps in PSUM. Bias+ReLU fused on ScalarE.

Sharding: data-parallel over batch, 2 images per core across 8 cores.
The channel gather/scatter are permutations of 512 channels (disjoint
blocks), applied on host as pure relabeling; all arithmetic (conv, bias,
relu) runs on device. If blocks_out ever contains duplicates (scatter-add
semantics with actual collisions) we fall back to a numpy implementation.
"""

import numpy as np
from contextlib import ExitStack

import concourse.bass as bass
import concourse.tile as tile
from concourse import bacc, mybir
from concourse.bass_utils import run_bass_kernel_spmd

# Problem shape (hardcoded per contract).
B, H, W = 16, 64, 64
C = 512
NB, CIN_B, COUT_B = 16, 32, 32
KS = 3
N_CORES = 8
BPC = B // N_CORES          # images per core
HP, WP = H + 2, W + 2       # zero-padded input plane
SPAT_P = BPC * HP * WP      # padded spatial per core
SPAT_O = BPC * H * W        # output spatial per core
NCT = C // 128              # 128-channel tiles
GPT = 128 // CIN_B          # groups per channel tile

F32 = mybir.dt.float32
MM_DT = mybir.dt.float32r   # fp32 bits streamed in fast mode (1 cyc/row @ N>=256)

_NC_CACHE = {}


def _build_nc(loop_k=1):
    nc = bacc.Bacc(None, target_bir_lowering=False)
    xt_d = nc.dram_tensor("xt", [C, BPC, HP, WP], MM_DT, kind="ExternalInput")
    wt_d = nc.dram_tensor("wt", [KS * KS, NCT, 128, 128], MM_DT, kind="ExternalInput")
    bg_d = nc.dram_tensor("bg", [NCT, 128, 1], F32, kind="ExternalInput")
    yt_d = nc.dram_tensor("yt", [C, SPAT_O], F32, kind="ExternalOutput")

    with ExitStack() as ctx:
        tc = ctx.enter_context(tile.TileContext(nc))
        xpool = ctx.enter_context(tc.tile_pool(name="x", bufs=1))
        wpool = ctx.enter_context(tc.tile_pool(name="w", bufs=1))
        bpool = ctx.enter_context(tc.tile_pool(name="b", bufs=1))
        ypool = ctx.enter_context(tc.tile_pool(name="y", bufs=4))
        pspool = ctx.enter_context(
            tc.tile_pool(name="ps", bufs=8, space=bass.MemorySpace.PSUM)
        )

        # Weights: 9 taps x 4 ctiles of [128,128], one SBUF tile, per-tap DMAs
        # (so the first matmul only waits for its own tap, not the whole load).
        w_sb = wpool.tile([128, KS * KS * NCT * 128], MM_DT, tag="wsb")
        for t in range(KS * KS):
            nc.sync.dma_start(
                w_sb[:, t * NCT * 128 : (t + 1) * NCT * 128].rearrange(
                    "p (c m) -> p c m", c=NCT
                ),
                wt_d[t].rearrange("c p m -> p c m"),
            )

        b_sb = bpool.tile([128, NCT], F32, tag="bsb")
        nc.gpsimd.dma_start(b_sb[:], bg_d[:].rearrange("c p o -> p (c o)"))

        def body():
            # x resident in SBUF: one tile per (image, ctile): [128, 66, 66],
            # loaded as 3 row-band DMAs so compute starts after the first band.
            x_sb = {}
            for b in range(BPC):
                for ct in range(NCT):
                    xt_tile = xpool.tile([128, HP, WP], MM_DT, tag=f"x{b}_{ct}")
                    for r0, r1 in ((0, 22), (22, 44), (44, HP)):
                        nc.sync.dma_start(
                            xt_tile[:, r0:r1, :],
                            xt_d[ct * 128 : (ct + 1) * 128, b, r0:r1],
                        )
                    x_sb[(b, ct)] = xt_tile

            ROWS_PER_CHUNK = 8  # 8 output rows x 64 cols = 512 = one PSUM bank
            n_chunks = H // ROWS_PER_CHUNK
            for b in range(BPC):
                for oc in range(n_chunks):
                    r0 = oc * ROWS_PER_CHUNK
                    for ct in range(NCT):
                        ps = pspool.tile([128, ROWS_PER_CHUNK, W], F32)
                        for t in range(KS * KS):
                            kh, kw = divmod(t, KS)
                            rhs = x_sb[(b, ct)][
                                :, r0 + kh : r0 + kh + ROWS_PER_CHUNK, kw : kw + W
                            ]
                            nc.tensor.matmul(
                                ps[:],
                                w_sb[:, bass.ts(t * NCT + ct, 128)],
                                rhs,
                                start=(t == 0),
                                stop=(t == KS * KS - 1),
                            )
                        y_sb = ypool.tile([128, ROWS_PER_CHUNK * W], F32)
                        nc.scalar.activation(
                            y_sb[:],
                            ps[:].rearrange("p a b -> p (a b)"),
                            mybir.ActivationFunctionType.Relu,
                            bias=b_sb[:, ct : ct + 1],
                        )
                        off = b * H * W + r0 * W
                        nc.sync.dma_start(
                            yt_d[
                                ct * 128 : (ct + 1) * 128,
                                off : off + ROWS_PER_CHUNK * W,
                            ],
                            y_sb[:],
                        )

        if loop_k == 1:
            body()
        else:
            with tc.For_i(0, loop_k, 1):
                body()
    nc.compile()
    return nc


def _get_nc():
    if "nc" not in _NC_CACHE:
        _NC_CACHE["nc"] = _build_nc()
    return _NC_CACHE["nc"]


def _numpy_fallback(x, weights, bias, blocks_in, blocks_out):
    bi = blocks_in.reshape(-1)
    bo = blocks_out.reshape(-1)
    xg = x[..., bi]  # [B,H,W,NB*CIN_B]
    xp = np.zeros((B, HP, WP, NB * CIN_B), np.float32)
    xp[:, 1 : H + 1, 1 : W + 1] = xg
    y = np.zeros((B, H, W, NB * COUT_B), np.float32)
    wg = weights.astype(np.float32)
    for g in range(NB):
        acc = np.zeros((B, H, W, COUT_B), np.float32)
        for kh in range(KS):
            for kw in range(KS):
                patch = xp[:, kh : kh + H, kw : kw + W, g * CIN_B : (g + 1) * CIN_B]
                acc += patch @ wg[g, kh, kw]
        y[..., g * COUT_B : (g + 1) * COUT_B] = acc
    out = np.zeros((B, H, W, C), np.float32)
    np.add.at(out, (slice(None), slice(None), slice(None), bo), y)
    out += bias.astype(np.float32)
    return np.maximum(out, 0.0)


def kernel(x, weights, bias, blocks_in, blocks_out):
    x = np.asarray(x, dtype=np.float32)
    weights = np.asarray(weights, dtype=np.float32)
    bias = np.asarray(bias, dtype=np.float32)
    bi = np.asarray(blocks_in).reshape(-1)
    bo = np.asarray(blocks_out).reshape(-1)

    if np.unique(bo).size != NB * COUT_B:
        # Actual scatter collisions: rare/never per setup_inputs; keep correct.
        return _numpy_fallback(x, weights, bias, blocks_in, blocks_out)

    # Host-side gather (pure relabel) + pad + channel-major layout.
    xg = np.moveaxis(x[..., bi], -1, 0)  # [512, B, H, W], grouped channels
    xt = np.zeros((C, B, HP, WP), np.float32)
    xt[:, :, 1 : H + 1, 1 : W + 1] = xg

    # Block-diagonal weight tiles [tap, ctile, 128, 128] (rows=cin, cols=cout).
    wt = np.zeros((KS * KS, NCT, 128, 128), np.float32)
    for g in range(NB):
        ct, j = divmod(g, GPT)
        wt[:, ct, j * CIN_B : (j + 1) * CIN_B, j * COUT_B : (j + 1) * COUT_B] = (
            weights[g].reshape(KS * KS, CIN_B, COUT_B)
        )

    bg = bias[bo].reshape(NCT, 128, 1).astype(np.float32)

    in_maps = []
    for k in range(N_CORES):
        shard = np.ascontiguousarray(xt[:, k * BPC : (k + 1) * BPC])
        in_maps.append({"xt": shard, "wt": wt, "bg": bg})

    global _LAST_IN_MAPS
    _LAST_IN_MAPS = in_maps
    nc = _get_nc()
    res = run_bass_kernel_spmd(nc, in_maps, list(range(N_CORES))).results

    # [512, B, H, W] grouped-channel output -> scatter (relabel) to out.
    y = np.concatenate(
        [res[k]["yt"].reshape(C, BPC, H, W) for k in range(N_CORES)], axis=1
    )
    out = np.empty((B, H, W, C), np.float32)
    out[..., bo] = np.moveaxis(y, 0, -1)
    return out


# revision 22
# speedup vs baseline: 1.3670x; 1.3670x over previous
"""BlockConv2D Trainium2 kernel.

Reference computation (see harness): gather 16 blocks of 32 input channels
(indices blocks_in) from x[16,64,64,512], run a per-block 3x3 'same' conv
(weights [16,3,3,32,32]), scatter-add the 16x32 output channels back to 512
channels (indices blocks_out), add bias, relu.

Mapping: groups are independent 32->32 channel convs. Four groups' 32x32
weight blocks pack into one 128x128 block-diagonal stationary operand, so
each tap of the conv is a single matmul per 128-channel tile:
    psum[co_tile, spatial] += W[tap, ctile].T @ x[ctile, spatial+shift]
accumulated over the 9 taps in PSUM. Bias+ReLU fused on ScalarE.

Sharding: data-parallel over batch, 2 images per core across 8 cores.
The channel gather/scatter are permutations of 512 channels (disjoint
blocks), applied on host as pure relabeling; all arithmetic (conv, bias,
relu) runs on device. If blocks_out ever contains duplicates (scatter-add
semantics with actual collisions) we fall back to a numpy implementation.
"""

import numpy as np
from contextlib import ExitStack

import concourse.bass as bass
import concourse.tile as tile
from concourse import bacc, mybir
from concourse.bass_utils import run_bass_kernel_spmd

# Problem shape (hardcoded per contract).
B, H, W = 16, 64, 64
C = 512
NB, CIN_B, COUT_B = 16, 32, 32
KS = 3
N_CORES = 8
BPC = B // N_CORES          # images per core
HP, WP = H + 2, W + 2       # zero-padded input plane
SPAT_P = BPC * HP * WP      # padded spatial per core
SPAT_O = BPC * H * W        # output spatial per core
NCT = C // 128              # 128-channel tiles
GPT = 128 // CIN_B          # groups per channel tile

F32 = mybir.dt.float32
BF16 = mybir.dt.bfloat16
MM_DT = mybir.dt.float32r   # fp32 bits streamed in fast mode (1 cyc/row @ N>=256)

# 's2d': width space-to-depth scheme (37.5% PE util, bf16 inputs)
# 'bd': block-diagonal scheme (25% PE util, float32r)
SCHEME = "s2d"

# s2d geometry: 4 output columns per stream column, 6 input positions,
# 16-channel ci chunks -> stationary [96, 128] per (group, kh, ci-chunk).
DW = 4                      # output cols packed per stream col
PW = 6                      # input w-positions in stationary rows
CC = 2                      # ci chunks of 16
CI_C = CIN_B // CC          # 16
WCOL = 17                   # w-s2d columns (padded W 68 = 4*17)
NPW = W // DW               # 16 output patches per row
HCH = 2                     # h chunks per image (32 rows x 16 patches = 512)

_NC_CACHE = {}


def _build_nc(loop_k=1):
    nc = bacc.Bacc(None, target_bir_lowering=False)
    xt_d = nc.dram_tensor("xt", [C, BPC, HP, WP], MM_DT, kind="ExternalInput")
    wt_d = nc.dram_tensor("wt", [KS * KS, NCT, 128, 128], MM_DT, kind="ExternalInput")
    bg_d = nc.dram_tensor("bg", [NCT, 128, 1], F32, kind="ExternalInput")
    yt_d = nc.dram_tensor("yt", [C, SPAT_O], F32, kind="ExternalOutput")

    with ExitStack() as ctx:
        tc = ctx.enter_context(tile.TileContext(nc))
        xpool = ctx.enter_context(tc.tile_pool(name="x", bufs=1))
        wpool = ctx.enter_context(tc.tile_pool(name="w", bufs=1))
        bpool = ctx.enter_context(tc.tile_pool(name="b", bufs=1))
        ypool = ctx.enter_context(tc.tile_pool(name="y", bufs=4))
        pspool = ctx.enter_context(
            tc.tile_pool(name="ps", bufs=8, space=bass.MemorySpace.PSUM)
        )

        # Weights: 9 taps x 4 ctiles of [128,128], one SBUF tile, per-tap DMAs
        # (so the first matmul only waits for its own tap, not the whole load).
        w_sb = wpool.tile([128, KS * KS * NCT * 128], MM_DT, tag="wsb")
        for t in range(KS * KS):
            nc.sync.dma_start(
                w_sb[:, t * NCT * 128 : (t + 1) * NCT * 128].rearrange(
                    "p (c m) -> p c m", c=NCT
                ),
                wt_d[t].rearrange("c p m -> p c m"),
            )

        b_sb = bpool.tile([128, NCT], F32, tag="bsb")
        nc.gpsimd.dma_start(b_sb[:], bg_d[:].rearrange("c p o -> p (c o)"))

        def body():
            # x resident in SBUF: one tile per (image, ctile): [128, 66, 66],
            # loaded as 3 row-band DMAs so compute starts after the first band.
            x_sb = {}
            for b in range(BPC):
                for ct in range(NCT):
                    xt_tile = xpool.tile([128, HP, WP], MM_DT, tag=f"x{b}_{ct}")
                    for r0, r1 in ((0, 22), (22, 44), (44, HP)):
                        nc.sync.dma_start(
                            xt_tile[:, r0:r1, :],
                            xt_d[ct * 128 : (ct + 1) * 128, b, r0:r1],
                        )
                    x_sb[(b, ct)] = xt_tile

            ROWS_PER_CHUNK = 8  # 8 output rows x 64 cols = 512 = one PSUM bank
            n_chunks = H // ROWS_PER_CHUNK
            for b in range(BPC):
                for oc in range(n_chunks):
                    r0 = oc * ROWS_PER_CHUNK
                    for ct in range(NCT):
                        ps = pspool.tile([128, ROWS_PER_CHUNK, W], F32)
                        for t in range(KS * KS):
                            kh, kw = divmod(t, KS)
                            rhs = x_sb[(b, ct)][
                                :, r0 + kh : r0 + kh + ROWS_PER_CHUNK, kw : kw + W
                            ]
                            nc.tensor.matmul(
                                ps[:],
                                w_sb[:, bass.ts(t * NCT + ct, 128)],
                                rhs,
                                start=(t == 0),
                                stop=(t == KS * KS - 1),
                            )
                        y_sb = ypool.tile([128, ROWS_PER_CHUNK * W], F32)
                        nc.scalar.activation(
                            y_sb[:],
                            ps[:].rearrange("p a b -> p (a b)"),
                            mybir.ActivationFunctionType.Relu,
                            bias=b_sb[:, ct : ct + 1],
                        )
                        off = b * H * W + r0 * W
                        nc.sync.dma_start(
                            yt_d[
                                ct * 128 : (ct + 1) * 128,
                                off : off + ROWS_PER_CHUNK * W,
                            ],
                            y_sb[:],
                        )

        if loop_k == 1:
            body()
        else:
            with tc.For_i(0, loop_k, 1):
                body()
    nc.compile()
    return nc


def _build_nc_s2d(loop_k=1):
    """Width space-to-depth grouped conv.

    Stream column = (h, pw): 4 output pixels w=4*pw+d, d<4, of one group.
    Stationary [96=(jr<6, ci<16), 128=(d<4, co<32)] holds w[g,kh,jr-d,ci,co]
    (kw = jr-d in 0..2). rhs rows jr carry x at padded w = 4*pw+jr, i.e.
    w-subgrid jr%4 shifted by jr//4 columns — host pre-assembles the 6-row
    tiles (1.5x input replication, bf16). kh accumulates in PSUM (3 passes
    x 2 ci chunks = 6 matmuls per 512-col PSUM bank).
    """
    nc = bacc.Bacc(None, target_bir_lowering=False)
    xt_d = nc.dram_tensor(
        "xt", [BPC, NB, CC, PW * CI_C, HP, WCOL], BF16, kind="ExternalInput"
    )
    wt_d = nc.dram_tensor(
        "wt", [NB, KS, CC, PW * CI_C, 128], BF16, kind="ExternalInput"
    )
    bg_d = nc.dram_tensor("bg", [NB, 128, 1], F32, kind="ExternalInput")
    yt_d = nc.dram_tensor(
        "yt", [NB, BPC, 128, HCH, H // HCH * NPW], BF16, kind="ExternalOutput"
    )

    with ExitStack() as ctx:
        tc = ctx.enter_context(tile.TileContext(nc))
        xpool = ctx.enter_context(tc.tile_pool(name="x", bufs=10))
        wpool = ctx.enter_context(tc.tile_pool(name="w", bufs=1))
        bpool = ctx.enter_context(tc.tile_pool(name="b", bufs=1))
        ypool = ctx.enter_context(tc.tile_pool(name="y", bufs=6))
        pspool = ctx.enter_context(
            tc.tile_pool(name="ps", bufs=8, space=bass.MemorySpace.PSUM)
        )

        # Stationaries resident: [96, NB*KS*CC*128] bf16; per-group DMAs
        # are issued lazily inside the body so group 0 compute starts early.
        wn = KS * CC * 128
        w_sb = wpool.tile([PW * CI_C, NB * wn], BF16, tag="wsb")
        w_loaded = set()

        def load_w(g):
            if g in w_loaded:
                return
            w_loaded.add(g)
            nc.sync.dma_start(
                w_sb[:, g * wn : (g + 1) * wn].rearrange(
                    "p (a c m) -> p a c m", a=KS, c=CC
                ),
                wt_d[g].rearrange("a c p m -> p a c m"),
            )

        b_sb = bpool.tile([128, NB], F32, tag="bsb")
        nc.sync.dma_start(b_sb[:], bg_d[:].rearrange("g p o -> p (g o)"))

        def body():
            for b in range(BPC):
                for g in range(NB):
                    load_w(g)
                    load_w(min(g + 1, NB - 1))
                    load_w(min(g + 2, NB - 1))
                    xt = {}
                    for cc in range(CC):
                        t = xpool.tile([PW * CI_C, HP, WCOL], BF16, tag=f"xc{cc}")
                        nc.sync.dma_start(t[:], xt_d[b, g, cc])
                        xt[cc] = t
                    for hc in range(HCH):
                        h0 = hc * (H // HCH)
                        ps = pspool.tile([128, H // HCH, NPW], F32)
                        first, last = (0, 0), (KS - 1, CC - 1)
                        for kh in range(KS):
                            for cc in range(CC):
                                rhs = xt[cc][
                                    :, h0 + kh : h0 + kh + H // HCH, 0:NPW
                                ]
                                off = (g * KS + kh) * CC + cc
                                nc.tensor.matmul(
                                    ps[:],
                                    w_sb[:, bass.ts(off, 128)],
                                    rhs,
                                    start=((kh, cc) == first),
                                    stop=((kh, cc) == last),
                                )
                        y_sb = ypool.tile([128, H // HCH * NPW], BF16)
                        nc.scalar.activation(
                            y_sb[:],
                            ps[:].rearrange("p a b -> p (a b)"),
                            mybir.ActivationFunctionType.Relu,
                            bias=b_sb[:, g : g + 1],
                        )
                        nc.sync.dma_start(yt_d[g, b, :, hc], y_sb[:])

        if loop_k == 1:
            body()
        else:
            with tc.For_i(0, loop_k, 1):
                body()
    nc.compile()
    return nc


def _prep_s2d_inputs(xg, weights_g, bias_bo):
    """Host assembly for the s2d scheme.

    xg: [C, B, H, W] gathered grouped channels (fp32).
    weights_g: [NB, KS, KS, CIN_B, COUT_B] fp32.
    bias_bo: [C] bias in grouped-output order.
    Returns (xt, wt, bg) host arrays matching _build_nc_s2d's dram decls.
    """
    import ml_dtypes

    bf = ml_dtypes.bfloat16
    # Pad: h -> 66 (1 each side), w -> 68 = 4*17 (1 left, 3 right).
    xp = np.zeros((C, B, HP, DW * WCOL), np.float32)
    xp[:, :, 1 : H + 1, 1 : W + 1] = xg
    xs = xp.reshape(C, B, HP, WCOL, DW)  # [...,col,j]: w_pad = 4*col + j

    xt = np.empty((B, NB, CC, PW * CI_C, HP, WCOL), bf)
    for jr in range(PW):
        sub, sh = jr % DW, jr // DW
        # partition rows jr*CI_C..(jr+1)*CI_C: subgrid sub shifted sh cols
        src = np.zeros((C, B, HP, WCOL), np.float32)
        if sh == 0:
            src[:] = xs[:, :, :, :, sub]
        else:
            src[:, :, :, : WCOL - sh] = xs[:, :, :, sh:, sub]
        # src[c] for channel c: map to (g, cc, ci)
        s = src.reshape(NB, CC, CI_C, B, HP, WCOL)
        xt[:, :, :, jr * CI_C : (jr + 1) * CI_C] = s.transpose(3, 0, 1, 2, 4, 5)

    wt = np.zeros((NB, KS, CC, PW * CI_C, 128), np.float32)
    for jr in range(PW):
        for d in range(DW):
            kw = jr - d
            if 0 <= kw < KS:
                # [NB, KS(kh), CC*CI_C(ci), CO]
                wsrc = weights_g[:, :, kw].reshape(NB, KS, CC, CI_C, COUT_B)
                wt[:, :, :, jr * CI_C : (jr + 1) * CI_C,
                   d * COUT_B : (d + 1) * COUT_B] = wsrc
    wt = wt.astype(bf)

    # bias per partition (d, co) for group g: bias_bo[g*32+co], same all d.
    bg = np.broadcast_to(
        bias_bo.reshape(NB, 1, COUT_B), (NB, DW, COUT_B)
    ).reshape(NB, 128, 1).astype(np.float32)
    return xt, wt, np.ascontiguousarray(bg)


def _unpack_s2d_output(yt_all):
    """yt_all: [NB, B, 128, HCH, (H//HCH)*NPW] -> [B, H, W, C] grouped fp32."""
    yt_all = np.asarray(yt_all, dtype=np.float32)
    y = yt_all.reshape(NB, B, DW, COUT_B, HCH, H // HCH, NPW)
    # -> [B, h(=HCH*H//HCH), w(=NPW*DW), NB, COUT_B]
    y = y.transpose(1, 4, 5, 6, 2, 0, 3)  # [B, HCH, h', pw, d, NB, co]
    y = y.reshape(B, H, W, NB * COUT_B)
    return y


def _get_nc():
    if "nc" not in _NC_CACHE:
        _NC_CACHE["nc"] = (
            _build_nc_s2d() if SCHEME == "s2d" else _build_nc()
        )
    return _NC_CACHE["nc"]


def _build_timed(loop_k):
    return _build_nc_s2d(loop_k) if SCHEME == "s2d" else _build_nc(loop_k)


def _numpy_fallback(x, weights, bias, blocks_in, blocks_out):
    bi = blocks_in.reshape(-1)
    bo = blocks_out.reshape(-1)
    xg = x[..., bi]  # [B,H,W,NB*CIN_B]
    xp = np.zeros((B, HP, WP, NB * CIN_B), np.float32)
    xp[:, 1 : H + 1, 1 : W + 1] = xg
    y = np.zeros((B, H, W, NB * COUT_B), np.float32)
    wg = weights.astype(np.float32)
    for g in range(NB):
        acc = np.zeros((B, H, W, COUT_B), np.float32)
        for kh in range(KS):
            for kw in range(KS):
                patch = xp[:, kh : kh + H, kw : kw + W, g * CIN_B : (g + 1) * CIN_B]
                acc += patch @ wg[g, kh, kw]
        y[..., g * COUT_B : (g + 1) * COUT_B] = acc
    out = np.zeros((B, H, W, C), np.float32)
    np.add.at(out, (slice(None), slice(None), slice(None), bo), y)
    out += bias.astype(np.float32)
    return np.maximum(out, 0.0)


def kernel(x, weights, bias, blocks_in, blocks_out):
    x = np.asarray(x, dtype=np.float32)
    weights = np.asarray(weights, dtype=np.float32)
    bias = np.asarray(bias, dtype=np.float32)
    bi = np.asarray(blocks_in).reshape(-1)
    bo = np.asarray(blocks_out).reshape(-1)

    if np.unique(bo).size != NB * COUT_B:
        # Actual scatter collisions: rare/never per setup_inputs; keep correct.
        return _numpy_fallback(x, weights, bias, blocks_in, blocks_out)

    # Host-side gather (pure relabel) + pad + channel-major layout.
    xg = np.moveaxis(x[..., bi], -1, 0)  # [512, B, H, W], grouped channels

    global _LAST_IN_MAPS
    if SCHEME == "s2d":
        xt, wt, bg = _prep_s2d_inputs(xg, weights, bias[bo])
        in_maps = [
            {
                "xt": np.ascontiguousarray(xt[k * BPC : (k + 1) * BPC]),
                "wt": wt,
                "bg": bg,
            }
            for k in range(N_CORES)
        ]
        _LAST_IN_MAPS = in_maps
        nc = _get_nc()
        res = run_bass_kernel_spmd(nc, in_maps, list(range(N_CORES))).results
        yt_all = np.concatenate(
            [res[k]["yt"] for k in range(N_CORES)], axis=1
        )  # [NB, B, 128, HCH, ...]
        yg = _unpack_s2d_output(yt_all)  # [B, H, W, C] grouped
        out = np.empty((B, H, W, C), np.float32)
        out[..., bo] = yg
        return out

    xt = np.zeros((C, B, HP, WP), np.float32)
    xt[:, :, 1 : H + 1, 1 : W + 1] = xg

    # Block-diagonal weight tiles [tap, ctile, 128, 128] (rows=cin, cols=cout).
    wt = np.zeros((KS * KS, NCT, 128, 128), np.float32)
    for g in range(NB):
        ct, j = divmod(g, GPT)
        wt[:, ct, j * CIN_B : (j + 1) * CIN_B, j * COUT_B : (j + 1) * COUT_B] = (
            weights[g].reshape(KS * KS, CIN_B, COUT_B)
        )

    bg = bias[bo].reshape(NCT, 128, 1).astype(np.float32)

    in_maps = []
    for k in range(N_CORES):
        shard = np.ascontiguousarray(xt[:, k * BPC : (k + 1) * BPC])
        in_maps.append({"xt": shard, "wt": wt, "bg": bg})

    _LAST_IN_MAPS = in_maps
    nc = _get_nc()
    res = run_bass_kernel_spmd(nc, in_maps, list(range(N_CORES))).results

    # [512, B, H, W] grouped-channel output -> scatter (relabel) to out.
    y = np.concatenate(
        [res[k]["yt"].reshape(C, BPC, H, W) for k in range(N_CORES)], axis=1
    )
    out = np.empty((B, H, W, C), np.float32)
    out[..., bo] = np.moveaxis(y, 0, -1)
    return out


# revision 23
# speedup vs baseline: 1.6280x; 1.1909x over previous
"""BlockConv2D Trainium2 kernel.

Reference computation (see harness): gather 16 blocks of 32 input channels
(indices blocks_in) from x[16,64,64,512], run a per-block 3x3 'same' conv
(weights [16,3,3,32,32]), scatter-add the 16x32 output channels back to 512
channels (indices blocks_out), add bias, relu.

Mapping: groups are independent 32->32 channel convs. Four groups' 32x32
weight blocks pack into one 128x128 block-diagonal stationary operand, so
each tap of the conv is a single matmul per 128-channel tile:
    psum[co_tile, spatial] += W[tap, ctile].T @ x[ctile, spatial+shift]
accumulated over the 9 taps in PSUM. Bias+ReLU fused on ScalarE.

Sharding: data-parallel over batch, 2 images per core across 8 cores.
The channel gather/scatter are permutations of 512 channels (disjoint
blocks), applied on host as pure relabeling; all arithmetic (conv, bias,
relu) runs on device. If blocks_out ever contains duplicates (scatter-add
semantics with actual collisions) we fall back to a numpy implementation.
"""

import numpy as np
from contextlib import ExitStack

import concourse.bass as bass
import concourse.tile as tile
from concourse import bacc, mybir
from concourse.bass_utils import run_bass_kernel_spmd

# Problem shape (hardcoded per contract).
B, H, W = 16, 64, 64
C = 512
NB, CIN_B, COUT_B = 16, 32, 32
KS = 3
N_CORES = 8
BPC = B // N_CORES          # images per core
HP, WP = H + 2, W + 2       # zero-padded input plane
SPAT_P = BPC * HP * WP      # padded spatial per core
SPAT_O = BPC * H * W        # output spatial per core
NCT = C // 128              # 128-channel tiles
GPT = 128 // CIN_B          # groups per channel tile

F32 = mybir.dt.float32
BF16 = mybir.dt.bfloat16
MM_DT = mybir.dt.float32r   # fp32 bits streamed in fast mode (1 cyc/row @ N>=256)

# 's2d': width space-to-depth scheme (37.5% PE util, bf16 inputs)
# 'bd': block-diagonal scheme (25% PE util, float32r)
SCHEME = "s2d"

# s2d geometry: 4 output columns per stream column, 6 input positions,
# 16-channel ci chunks -> stationary [96, 128] per (group, kh, ci-chunk).
DW = 4                      # output cols packed per stream col
PW = 6                      # input w-positions in stationary rows
CC = 2                      # ci chunks of 16
CI_C = CIN_B // CC          # 16
WCOL = 17                   # w-s2d columns (padded W 68 = 4*17)
NPW = W // DW               # 16 output patches per row
HCH = 2                     # h chunks per image (32 rows x 16 patches = 512)

_NC_CACHE = {}


def _build_nc(loop_k=1):
    nc = bacc.Bacc(None, target_bir_lowering=False)
    xt_d = nc.dram_tensor("xt", [C, BPC, HP, WP], MM_DT, kind="ExternalInput")
    wt_d = nc.dram_tensor("wt", [KS * KS, NCT, 128, 128], MM_DT, kind="ExternalInput")
    bg_d = nc.dram_tensor("bg", [NCT, 128, 1], F32, kind="ExternalInput")
    yt_d = nc.dram_tensor("yt", [C, SPAT_O], F32, kind="ExternalOutput")

    with ExitStack() as ctx:
        tc = ctx.enter_context(tile.TileContext(nc))
        xpool = ctx.enter_context(tc.tile_pool(name="x", bufs=1))
        wpool = ctx.enter_context(tc.tile_pool(name="w", bufs=1))
        bpool = ctx.enter_context(tc.tile_pool(name="b", bufs=1))
        ypool = ctx.enter_context(tc.tile_pool(name="y", bufs=4))
        pspool = ctx.enter_context(
            tc.tile_pool(name="ps", bufs=8, space=bass.MemorySpace.PSUM)
        )

        # Weights: 9 taps x 4 ctiles of [128,128], one SBUF tile, per-tap DMAs
        # (so the first matmul only waits for its own tap, not the whole load).
        w_sb = wpool.tile([128, KS * KS * NCT * 128], MM_DT, tag="wsb")
        for t in range(KS * KS):
            nc.sync.dma_start(
                w_sb[:, t * NCT * 128 : (t + 1) * NCT * 128].rearrange(
                    "p (c m) -> p c m", c=NCT
                ),
                wt_d[t].rearrange("c p m -> p c m"),
            )

        b_sb = bpool.tile([128, NCT], F32, tag="bsb")
        nc.gpsimd.dma_start(b_sb[:], bg_d[:].rearrange("c p o -> p (c o)"))

        def body():
            # x resident in SBUF: one tile per (image, ctile): [128, 66, 66],
            # loaded as 3 row-band DMAs so compute starts after the first band.
            x_sb = {}
            for b in range(BPC):
                for ct in range(NCT):
                    xt_tile = xpool.tile([128, HP, WP], MM_DT, tag=f"x{b}_{ct}")
                    for r0, r1 in ((0, 22), (22, 44), (44, HP)):
                        nc.sync.dma_start(
                            xt_tile[:, r0:r1, :],
                            xt_d[ct * 128 : (ct + 1) * 128, b, r0:r1],
                        )
                    x_sb[(b, ct)] = xt_tile

            ROWS_PER_CHUNK = 8  # 8 output rows x 64 cols = 512 = one PSUM bank
            n_chunks = H // ROWS_PER_CHUNK
            for b in range(BPC):
                for oc in range(n_chunks):
                    r0 = oc * ROWS_PER_CHUNK
                    for ct in range(NCT):
                        ps = pspool.tile([128, ROWS_PER_CHUNK, W], F32)
                        for t in range(KS * KS):
                            kh, kw = divmod(t, KS)
                            rhs = x_sb[(b, ct)][
                                :, r0 + kh : r0 + kh + ROWS_PER_CHUNK, kw : kw + W
                            ]
                            nc.tensor.matmul(
                                ps[:],
                                w_sb[:, bass.ts(t * NCT + ct, 128)],
                                rhs,
                                start=(t == 0),
                                stop=(t == KS * KS - 1),
                            )
                        y_sb = ypool.tile([128, ROWS_PER_CHUNK * W], F32)
                        nc.scalar.activation(
                            y_sb[:],
                            ps[:].rearrange("p a b -> p (a b)"),
                            mybir.ActivationFunctionType.Relu,
                            bias=b_sb[:, ct : ct + 1],
                        )
                        off = b * H * W + r0 * W
                        nc.sync.dma_start(
                            yt_d[
                                ct * 128 : (ct + 1) * 128,
                                off : off + ROWS_PER_CHUNK * W,
                            ],
                            y_sb[:],
                        )

        if loop_k == 1:
            body()
        else:
            with tc.For_i(0, loop_k, 1):
                body()
    nc.compile()
    return nc


def _build_nc_s2d(loop_k=1):
    """Width space-to-depth grouped conv.

    Stream column = (h, pw): 4 output pixels w=4*pw+d, d<4, of one group.
    Stationary [96=(jr<6, ci<16), 128=(d<4, co<32)] holds w[g,kh,jr-d,ci,co]
    (kw = jr-d in 0..2). rhs rows jr carry x at padded w = 4*pw+jr, i.e.
    w-subgrid jr%4 shifted by jr//4 columns — host pre-assembles the 6-row
    tiles (1.5x input replication, bf16). kh accumulates in PSUM (3 passes
    x 2 ci chunks = 6 matmuls per 512-col PSUM bank).
    """
    nc = bacc.Bacc(None, target_bir_lowering=False, num_swdge_queues=4)
    xt_d = nc.dram_tensor(
        "xt", [BPC, NB, CC, PW * CI_C, HP, WCOL], BF16, kind="ExternalInput"
    )
    wt_d = nc.dram_tensor(
        "wt", [NB, KS, CC, PW * CI_C, 128], BF16, kind="ExternalInput"
    )
    bg_d = nc.dram_tensor("bg", [NB, 128, 1], F32, kind="ExternalInput")
    yt_d = nc.dram_tensor(
        "yt", [NB, BPC, 128, HCH, H // HCH * NPW], BF16, kind="ExternalOutput"
    )

    with ExitStack() as ctx:
        tc = ctx.enter_context(tile.TileContext(nc))
        xpool = ctx.enter_context(tc.tile_pool(name="x", bufs=10))
        wpool = ctx.enter_context(tc.tile_pool(name="w", bufs=1))
        bpool = ctx.enter_context(tc.tile_pool(name="b", bufs=1))
        ypool = ctx.enter_context(tc.tile_pool(name="y", bufs=6))
        pspool = ctx.enter_context(
            tc.tile_pool(name="ps", bufs=8, space=bass.MemorySpace.PSUM)
        )

        # Stationaries resident: [96, NB*KS*CC*128] bf16; per-group DMAs
        # are issued lazily inside the body so group 0 compute starts early.
        wn = KS * CC * 128
        w_sb = wpool.tile([PW * CI_C, NB * wn], BF16, tag="wsb")
        w_loaded = set()

        def load_w(g):
            if g in w_loaded:
                return
            w_loaded.add(g)
            nc.sync.dma_start(
                w_sb[:, g * wn : (g + 1) * wn].rearrange(
                    "p (a c m) -> p a c m", a=KS, c=CC
                ),
                wt_d[g].rearrange("a c p m -> p a c m"),
            )

        b_sb = bpool.tile([128, NB], F32, tag="bsb")
        nc.sync.dma_start(b_sb[:], bg_d[:].rearrange("g p o -> p (g o)"))

        def body():
            for b in range(BPC):
                for g in range(NB):
                    load_w(g)
                    load_w(min(g + 1, NB - 1))
                    load_w(min(g + 2, NB - 1))
                    xt = {}
                    for cc in range(CC):
                        t = xpool.tile([PW * CI_C, HP, WCOL], BF16, tag=f"xc{cc}")
                        nc.sync.dma_start(t[:], xt_d[b, g, cc])
                        xt[cc] = t
                    for hc in range(HCH):
                        h0 = hc * (H // HCH)
                        ps = pspool.tile([128, H // HCH, NPW], F32)
                        first, last = (0, 0), (KS - 1, CC - 1)
                        for kh in range(KS):
                            for cc in range(CC):
                                rhs = xt[cc][
                                    :, h0 + kh : h0 + kh + H // HCH, 0:NPW
                                ]
                                off = (g * KS + kh) * CC + cc
                                nc.tensor.matmul(
                                    ps[:],
                                    w_sb[:, bass.ts(off, 128)],
                                    rhs,
                                    start=((kh, cc) == first),
                                    stop=((kh, cc) == last),
                                )
                        y_sb = ypool.tile([128, H // HCH * NPW], BF16)
                        nc.scalar.activation(
                            y_sb[:],
                            ps[:].rearrange("p a b -> p (a b)"),
                            mybir.ActivationFunctionType.Relu,
                            bias=b_sb[:, g : g + 1],
                        )
                        nc.gpsimd.dma_start(yt_d[g, b, :, hc], y_sb[:])

        if loop_k == 1:
            body()
        else:
            with tc.For_i(0, loop_k, 1):
                body()
    nc.compile()
    return nc


def _prep_s2d_inputs(xg, weights_g, bias_bo):
    """Host assembly for the s2d scheme.

    xg: [C, B, H, W] gathered grouped channels (fp32).
    weights_g: [NB, KS, KS, CIN_B, COUT_B] fp32.
    bias_bo: [C] bias in grouped-output order.
    Returns (xt, wt, bg) host arrays matching _build_nc_s2d's dram decls.
    """
    import ml_dtypes

    bf = ml_dtypes.bfloat16
    # Pad: h -> 66 (1 each side), w -> 68 = 4*17 (1 left, 3 right).
    xp = np.zeros((C, B, HP, DW * WCOL), np.float32)
    xp[:, :, 1 : H + 1, 1 : W + 1] = xg
    xs = xp.reshape(C, B, HP, WCOL, DW)  # [...,col,j]: w_pad = 4*col + j

    xt = np.empty((B, NB, CC, PW * CI_C, HP, WCOL), bf)
    for jr in range(PW):
        sub, sh = jr % DW, jr // DW
        # partition rows jr*CI_C..(jr+1)*CI_C: subgrid sub shifted sh cols
        src = np.zeros((C, B, HP, WCOL), np.float32)
        if sh == 0:
            src[:] = xs[:, :, :, :, sub]
        else:
            src[:, :, :, : WCOL - sh] = xs[:, :, :, sh:, sub]
        # src[c] for channel c: map to (g, cc, ci)
        s = src.reshape(NB, CC, CI_C, B, HP, WCOL)
        xt[:, :, :, jr * CI_C : (jr + 1) * CI_C] = s.transpose(3, 0, 1, 2, 4, 5)

    wt = np.zeros((NB, KS, CC, PW * CI_C, 128), np.float32)
    for jr in range(PW):
        for d in range(DW):
            kw = jr - d
            if 0 <= kw < KS:
                # [NB, KS(kh), CC*CI_C(ci), CO]
                wsrc = weights_g[:, :, kw].reshape(NB, KS, CC, CI_C, COUT_B)
                wt[:, :, :, jr * CI_C : (jr + 1) * CI_C,
                   d * COUT_B : (d + 1) * COUT_B] = wsrc
    wt = wt.astype(bf)

    # bias per partition (d, co) for group g: bias_bo[g*32+co], same all d.
    bg = np.broadcast_to(
        bias_bo.reshape(NB, 1, COUT_B), (NB, DW, COUT_B)
    ).reshape(NB, 128, 1).astype(np.float32)
    return xt, wt, np.ascontiguousarray(bg)


def _unpack_s2d_output(yt_all):
    """yt_all: [NB, B, 128, HCH, (H//HCH)*NPW] -> [B, H, W, C] grouped fp32."""
    yt_all = np.asarray(yt_all, dtype=np.float32)
    y = yt_all.reshape(NB, B, DW, COUT_B, HCH, H // HCH, NPW)
    # -> [B, h(=HCH*H//HCH), w(=NPW*DW), NB, COUT_B]
    y = y.transpose(1, 4, 5, 6, 2, 0, 3)  # [B, HCH, h', pw, d, NB, co]
    y = y.reshape(B, H, W, NB * COUT_B)
    return y


def _get_nc():
    if "nc" not in _NC_CACHE:
        _NC_CACHE["nc"] = (
            _build_nc_s2d() if SCHEME == "s2d" else _build_nc()
        )
    return _NC_CACHE["nc"]


def _build_timed(loop_k):
    return _build_nc_s2d(loop_k) if SCHEME == "s2d" else _build_nc(loop_k)


def _numpy_fallback(x, weights, bias, blocks_in, blocks_out):
    bi = blocks_in.reshape(-1)
    bo = blocks_out.reshape(-1)
    xg = x[..., bi]  # [B,H,W,NB*CIN_B]
    xp = np.zeros((B, HP, WP, NB * CIN_B), np.float32)
    xp[:, 1 : H + 1, 1 : W + 1] = xg
    y = np.zeros((B, H, W, NB * COUT_B), np.float32)
    wg = weights.astype(np.float32)
    for g in range(NB):
        acc = np.zeros((B, H, W, COUT_B), np.float32)
        for kh in range(KS):
            for kw in range(KS):
                patch = xp[:, kh : kh + H, kw : kw + W, g * CIN_B : (g + 1) * CIN_B]
                acc += patch @ wg[g, kh, kw]
        y[..., g * COUT_B : (g + 1) * COUT_B] = acc
    out = np.zeros((B, H, W, C), np.float32)
    np.add.at(out, (slice(None), slice(None), slice(None), bo), y)
    out += bias.astype(np.float32)
    return np.maximum(out, 0.0)


def kernel(x, weights, bias, blocks_in, blocks_out):
    x = np.asarray(x, dtype=np.float32)
    weights = np.asarray(weights, dtype=np.float32)
    bias = np.asarray(bias, dtype=np.float32)
    bi = np.asarray(blocks_in).reshape(-1)
    bo = np.asarray(blocks_out).reshape(-1)

    if np.unique(bo).size != NB * COUT_B:
        # Actual scatter collisions: rare/never per setup_inputs; keep correct.
        return _numpy_fallback(x, weights, bias, blocks_in, blocks_out)

    # Host-side gather (pure relabel) + pad + channel-major layout.
    xg = np.moveaxis(x[..., bi], -1, 0)  # [512, B, H, W], grouped channels

    global _LAST_IN_MAPS
    if SCHEME == "s2d":
        xt, wt, bg = _prep_s2d_inputs(xg, weights, bias[bo])
        in_maps = [
            {
                "xt": np.ascontiguousarray(xt[k * BPC : (k + 1) * BPC]),
                "wt": wt,
                "bg": bg,
            }
            for k in range(N_CORES)
        ]
        _LAST_IN_MAPS = in_maps
        nc = _get_nc()
        res = run_bass_kernel_spmd(nc, in_maps, list(range(N_CORES))).results
        yt_all = np.concatenate(
            [res[k]["yt"] for k in range(N_CORES)], axis=1
        )  # [NB, B, 128, HCH, ...]
        yg = _unpack_s2d_output(yt_all)  # [B, H, W, C] grouped
        out = np.empty((B, H, W, C), np.float32)
        out[..., bo] = yg
        return out

    xt = np.zeros((C, B, HP, WP), np.float32)
    xt[:, :, 1 : H + 1, 1 : W + 1] = xg

    # Block-diagonal weight tiles [tap, ctile, 128, 128] (rows=cin, cols=cout).
    wt = np.zeros((KS * KS, NCT, 128, 128), np.float32)
    for g in range(NB):
        ct, j = divmod(g, GPT)
        wt[:, ct, j * CIN_B : (j + 1) * CIN_B, j * COUT_B : (j + 1) * COUT_B] = (
            weights[g].reshape(KS * KS, CIN_B, COUT_B)
        )

    bg = bias[bo].reshape(NCT, 128, 1).astype(np.float32)

    in_maps = []
    for k in range(N_CORES):
        shard = np.ascontiguousarray(xt[:, k * BPC : (k + 1) * BPC])
        in_maps.append({"xt": shard, "wt": wt, "bg": bg})

    _LAST_IN_MAPS = in_maps
    nc = _get_nc()
    res = run_bass_kernel_spmd(nc, in_maps, list(range(N_CORES))).results

    # [512, B, H, W] grouped-channel output -> scatter (relabel) to out.
    y = np.concatenate(
        [res[k]["yt"].reshape(C, BPC, H, W) for k in range(N_CORES)], axis=1
    )
    out = np.empty((B, H, W, C), np.float32)
    out[..., bo] = np.moveaxis(y, 0, -1)
    return out


# revision 28
# speedup vs baseline: 2.0982x; 1.2888x over previous
"""BlockConv2D Trainium2 kernel.

Reference computation (see harness): gather 16 blocks of 32 input channels
(indices blocks_in) from x[16,64,64,512], run a per-block 3x3 'same' conv
(weights [16,3,3,32,32]), scatter-add the 16x32 output channels back to 512
channels (indices blocks_out), add bias, relu.

Mapping: groups are independent 32->32 channel convs. Four groups' 32x32
weight blocks pack into one 128x128 block-diagonal stationary operand, so
each tap of the conv is a single matmul per 128-channel tile:
    psum[co_tile, spatial] += W[tap, ctile].T @ x[ctile, spatial+shift]
accumulated over the 9 taps in PSUM. Bias+ReLU fused on ScalarE.

Sharding: data-parallel over batch, 2 images per core across 8 cores.
The channel gather/scatter are permutations of 512 channels (disjoint
blocks), applied on host as pure relabeling; all arithmetic (conv, bias,
relu) runs on device. If blocks_out ever contains duplicates (scatter-add
semantics with actual collisions) we fall back to a numpy implementation.
"""

import numpy as np
from contextlib import ExitStack

import concourse.bass as bass
import concourse.tile as tile
from concourse import bacc, mybir
from concourse.bass_utils import run_bass_kernel_spmd

# Problem shape (hardcoded per contract).
B, H, W = 16, 64, 64
C = 512
NB, CIN_B, COUT_B = 16, 32, 32
KS = 3
N_CORES = 8
BPC = B // N_CORES          # images per core
HP, WP = H + 2, W + 2       # zero-padded input plane
SPAT_P = BPC * HP * WP      # padded spatial per core
SPAT_O = BPC * H * W        # output spatial per core
NCT = C // 128              # 128-channel tiles
GPT = 128 // CIN_B          # groups per channel tile

F32 = mybir.dt.float32
BF16 = mybir.dt.bfloat16
MM_DT = mybir.dt.float32r   # fp32 bits streamed in fast mode (1 cyc/row @ N>=256)

# 's2d': width space-to-depth scheme (37.5% PE util, bf16 inputs)
# 'bd': block-diagonal scheme (25% PE util, float32r)
SCHEME = "s2d2"

# s2d geometry: 4 output columns per stream column, 6 input positions,
# 16-channel ci chunks -> stationary [96, 128] per (group, kh, ci-chunk).
DW = 4                      # output cols packed per stream col
PW = 6                      # input w-positions in stationary rows
CC = 2                      # ci chunks of 16
CI_C = CIN_B // CC          # 16
WCOL = 17                   # w-s2d columns (padded W 68 = 4*17)
NPW = W // DW               # 16 output patches per row
HCH = 2                     # h chunks per image (32 rows x 16 patches = 512)

_NC_CACHE = {}


def _build_nc(loop_k=1):
    nc = bacc.Bacc(None, target_bir_lowering=False)
    xt_d = nc.dram_tensor("xt", [C, BPC, HP, WP], MM_DT, kind="ExternalInput")
    wt_d = nc.dram_tensor("wt", [KS * KS, NCT, 128, 128], MM_DT, kind="ExternalInput")
    bg_d = nc.dram_tensor("bg", [NCT, 128, 1], F32, kind="ExternalInput")
    yt_d = nc.dram_tensor("yt", [C, SPAT_O], F32, kind="ExternalOutput")

    with ExitStack() as ctx:
        tc = ctx.enter_context(tile.TileContext(nc))
        xpool = ctx.enter_context(tc.tile_pool(name="x", bufs=1))
        wpool = ctx.enter_context(tc.tile_pool(name="w", bufs=1))
        bpool = ctx.enter_context(tc.tile_pool(name="b", bufs=1))
        ypool = ctx.enter_context(tc.tile_pool(name="y", bufs=4))
        pspool = ctx.enter_context(
            tc.tile_pool(name="ps", bufs=8, space=bass.MemorySpace.PSUM)
        )

        # Weights: 9 taps x 4 ctiles of [128,128], one SBUF tile, per-tap DMAs
        # (so the first matmul only waits for its own tap, not the whole load).
        w_sb = wpool.tile([128, KS * KS * NCT * 128], MM_DT, tag="wsb")
        for t in range(KS * KS):
            nc.sync.dma_start(
                w_sb[:, t * NCT * 128 : (t + 1) * NCT * 128].rearrange(
                    "p (c m) -> p c m", c=NCT
                ),
                wt_d[t].rearrange("c p m -> p c m"),
            )

        b_sb = bpool.tile([128, NCT], F32, tag="bsb")
        nc.gpsimd.dma_start(b_sb[:], bg_d[:].rearrange("c p o -> p (c o)"))

        def body():
            # x resident in SBUF: one tile per (image, ctile): [128, 66, 66],
            # loaded as 3 row-band DMAs so compute starts after the first band.
            x_sb = {}
            for b in range(BPC):
                for ct in range(NCT):
                    xt_tile = xpool.tile([128, HP, WP], MM_DT, tag=f"x{b}_{ct}")
                    for r0, r1 in ((0, 22), (22, 44), (44, HP)):
                        nc.sync.dma_start(
                            xt_tile[:, r0:r1, :],
                            xt_d[ct * 128 : (ct + 1) * 128, b, r0:r1],
                        )
                    x_sb[(b, ct)] = xt_tile

            ROWS_PER_CHUNK = 8  # 8 output rows x 64 cols = 512 = one PSUM bank
            n_chunks = H // ROWS_PER_CHUNK
            for b in range(BPC):
                for oc in range(n_chunks):
                    r0 = oc * ROWS_PER_CHUNK
                    for ct in range(NCT):
                        ps = pspool.tile([128, ROWS_PER_CHUNK, W], F32)
                        for t in range(KS * KS):
                            kh, kw = divmod(t, KS)
                            rhs = x_sb[(b, ct)][
                                :, r0 + kh : r0 + kh + ROWS_PER_CHUNK, kw : kw + W
                            ]
                            nc.tensor.matmul(
                                ps[:],
                                w_sb[:, bass.ts(t * NCT + ct, 128)],
                                rhs,
                                start=(t == 0),
                                stop=(t == KS * KS - 1),
                            )
                        y_sb = ypool.tile([128, ROWS_PER_CHUNK * W], F32)
                        nc.scalar.activation(
                            y_sb[:],
                            ps[:].rearrange("p a b -> p (a b)"),
                            mybir.ActivationFunctionType.Relu,
                            bias=b_sb[:, ct : ct + 1],
                        )
                        off = b * H * W + r0 * W
                        nc.sync.dma_start(
                            yt_d[
                                ct * 128 : (ct + 1) * 128,
                                off : off + ROWS_PER_CHUNK * W,
                            ],
                            y_sb[:],
                        )

        if loop_k == 1:
            body()
        else:
            with tc.For_i(0, loop_k, 1):
                body()
    nc.compile()
    return nc


def _build_nc_s2d(loop_k=1):
    """Width space-to-depth grouped conv.

    Stream column = (h, pw): 4 output pixels w=4*pw+d, d<4, of one group.
    Stationary [96=(jr<6, ci<16), 128=(d<4, co<32)] holds w[g,kh,jr-d,ci,co]
    (kw = jr-d in 0..2). rhs rows jr carry x at padded w = 4*pw+jr, i.e.
    w-subgrid jr%4 shifted by jr//4 columns — host pre-assembles the 6-row
    tiles (1.5x input replication, bf16). kh accumulates in PSUM (3 passes
    x 2 ci chunks = 6 matmuls per 512-col PSUM bank).
    """
    nc = bacc.Bacc(None, target_bir_lowering=False, num_swdge_queues=4)
    xt_d = nc.dram_tensor(
        "xt", [BPC, NB, CC, PW * CI_C, HP, WCOL], BF16, kind="ExternalInput"
    )
    wt_d = nc.dram_tensor(
        "wt", [NB, KS, CC, PW * CI_C, 128], BF16, kind="ExternalInput"
    )
    bg_d = nc.dram_tensor("bg", [NB, 128, 1], F32, kind="ExternalInput")
    yt_d = nc.dram_tensor(
        "yt", [NB, BPC, 128, HCH, H // HCH * NPW], BF16, kind="ExternalOutput"
    )

    with ExitStack() as ctx:
        tc = ctx.enter_context(tile.TileContext(nc))
        xpool = ctx.enter_context(tc.tile_pool(name="x", bufs=10))
        wpool = ctx.enter_context(tc.tile_pool(name="w", bufs=1))
        bpool = ctx.enter_context(tc.tile_pool(name="b", bufs=1))
        ypool = ctx.enter_context(tc.tile_pool(name="y", bufs=6))
        pspool = ctx.enter_context(
            tc.tile_pool(name="ps", bufs=8, space=bass.MemorySpace.PSUM)
        )

        # Stationaries resident: [96, NB*KS*CC*128] bf16; per-group DMAs
        # are issued lazily inside the body so group 0 compute starts early.
        wn = KS * CC * 128
        w_sb = wpool.tile([PW * CI_C, NB * wn], BF16, tag="wsb")
        w_loaded = set()

        def load_w(g):
            if g in w_loaded:
                return
            w_loaded.add(g)
            nc.sync.dma_start(
                w_sb[:, g * wn : (g + 1) * wn].rearrange(
                    "p (a c m) -> p a c m", a=KS, c=CC
                ),
                wt_d[g].rearrange("a c p m -> p a c m"),
            )

        b_sb = bpool.tile([128, NB], F32, tag="bsb")
        nc.sync.dma_start(b_sb[:], bg_d[:].rearrange("g p o -> p (g o)"))

        def body():
            for b in range(BPC):
                for g in range(NB):
                    load_w(g)
                    load_w(min(g + 1, NB - 1))
                    load_w(min(g + 2, NB - 1))
                    xt = {}
                    for cc in range(CC):
                        t = xpool.tile([PW * CI_C, HP, WCOL], BF16, tag=f"xc{cc}")
                        nc.sync.dma_start(t[:], xt_d[b, g, cc])
                        xt[cc] = t
                    for hc in range(HCH):
                        h0 = hc * (H // HCH)
                        ps = pspool.tile([128, H // HCH, NPW], F32)
                        first, last = (0, 0), (KS - 1, CC - 1)
                        for kh in range(KS):
                            for cc in range(CC):
                                rhs = xt[cc][
                                    :, h0 + kh : h0 + kh + H // HCH, 0:NPW
                                ]
                                off = (g * KS + kh) * CC + cc
                                nc.tensor.matmul(
                                    ps[:],
                                    w_sb[:, bass.ts(off, 128)],
                                    rhs,
                                    start=((kh, cc) == first),
                                    stop=((kh, cc) == last),
                                )
                        y_sb = ypool.tile([128, H // HCH * NPW], BF16)
                        nc.scalar.activation(
                            y_sb[:],
                            ps[:].rearrange("p a b -> p (a b)"),
                            mybir.ActivationFunctionType.Relu,
                            bias=b_sb[:, g : g + 1],
                        )
                        nc.gpsimd.dma_start(yt_d[g, b, :, hc], y_sb[:])

        if loop_k == 1:
            body()
        else:
            with tc.For_i(0, loop_k, 1, hint_engines=(mybir.EngineType.PE,)):
                body()
    nc.compile()
    return nc


def _build_nc_s2d2(loop_k=1):
    """2x2-patch space-to-depth grouped conv, 56.25% PE utilization.

    x is decomposed 2x2 (padded coords): subgrid (a,b)[lh,lw] = xpad[2lh+a,
    2lw+b]. A stream column is one 2x2 output patch (ph,pw); the four input
    positions it needs in each axis fold into parity a/b (partition dim) and
    shift s in {0,1} (a pure AP offset on the same tile). Four shift-matmuls
    (s_h,s_w), each [128=(a,b,ci=32), 128=(dh,dw,co)], cover all 9 taps:
    stationary cell ((a,b,ci),(dh,dw,co)) = w[g, 2*s_h+a-dh, 2*s_w+b-dw, ci,
    co] (zero outside 0..2). No input replication, full ci per pass.
    """
    nc = bacc.Bacc(None, target_bir_lowering=False, num_swdge_queues=4)
    LH = HP // 2  # 33 subgrid lines per axis
    xt_d = nc.dram_tensor("xt", [BPC, NB, 128, LH, LH], BF16, kind="ExternalInput")
    wt_d = nc.dram_tensor("wt", [NB, 2, 2, 128, 128], BF16, kind="ExternalInput")
    bg_d = nc.dram_tensor("bg", [NB, 128, 1], F32, kind="ExternalInput")
    # psum chunk = (ph 16, pw 32) = 512 cols; 2 chunks per image.
    yt_d = nc.dram_tensor(
        "yt", [NB, BPC, 128, 2, 512], BF16, kind="ExternalOutput"
    )

    with ExitStack() as ctx:
        tc = ctx.enter_context(tile.TileContext(nc))
        xpool = ctx.enter_context(tc.tile_pool(name="x", bufs=6))
        wpool = ctx.enter_context(tc.tile_pool(name="w", bufs=1))
        bpool = ctx.enter_context(tc.tile_pool(name="b", bufs=1))
        ypool = ctx.enter_context(tc.tile_pool(name="y", bufs=6))
        pspool = ctx.enter_context(
            tc.tile_pool(name="ps", bufs=8, space=bass.MemorySpace.PSUM)
        )

        w_sb = wpool.tile([128, NB * 4 * 128], BF16, tag="wsb")
        w_loaded = set()

        def load_w(g):
            if g in w_loaded:
                return
            w_loaded.add(g)
            nc.sync.dma_start(
                w_sb[:, g * 512 : (g + 1) * 512].rearrange(
                    "p (a b m) -> p a b m", a=2, b=2
                ),
                wt_d[g].rearrange("a b p m -> p a b m"),
            )

        b_sb = bpool.tile([128, NB], F32, tag="bsb")
        nc.sync.dma_start(b_sb[:], bg_d[:].rearrange("g p o -> p (g o)"))

        def body():
            for b in range(BPC):
                for g in range(NB):
                    load_w(g)
                    load_w(min(g + 1, NB - 1))
                    load_w(min(g + 2, NB - 1))
                    xt = xpool.tile([128, LH, LH], BF16, tag="xtile")
                    nc.sync.dma_start(xt[:], xt_d[b, g])
                    y_sb = ypool.tile([128, 2, 512], BF16)
                    for hc in range(2):
                        ps = pspool.tile([128, 16, 32], F32)
                        for sh in range(2):
                            for sw in range(2):
                                rhs = xt[
                                    :,
                                    hc * 16 + sh : hc * 16 + sh + 16,
                                    sw : sw + 32,
                                ]
                                nc.tensor.matmul(
                                    ps[:],
                                    w_sb[:, bass.ts(g * 4 + sh * 2 + sw, 128)],
                                    rhs,
                                    start=(sh == 0 and sw == 0),
                                    stop=(sh == 1 and sw == 1),
                                )
                        nc.scalar.activation(
                            y_sb[:, hc],
                            ps[:].rearrange("p a b -> p (a b)"),
                            mybir.ActivationFunctionType.Relu,
                            bias=b_sb[:, g : g + 1],
                        )
                    nc.gpsimd.dma_start(yt_d[g, b], y_sb[:])

        if loop_k == 1:
            body()
        else:
            with tc.For_i(0, loop_k, 1, hint_engines=(mybir.EngineType.PE,)):
                body()
    nc.compile()
    return nc


def _prep_s2d2_inputs(xg, weights_g, bias_bo):
    """Host assembly for the 2x2-patch s2d scheme.

    xg: [C, B, H, W] gathered grouped channels (fp32).
    Returns (xt, wt, bg) matching _build_nc_s2d2's dram decls.
    """
    import ml_dtypes

    bf = ml_dtypes.bfloat16
    LH = HP // 2
    xp = np.zeros((C, B, HP, HP), np.float32)
    xp[:, :, 1 : H + 1, 1 : W + 1] = xg
    # subgrid (a,b): xp[2lh+a, 2lw+b] -> [C, B, a, b, LH, LH]
    xs = xp.reshape(C, B, LH, 2, LH, 2).transpose(0, 1, 3, 5, 2, 4)
    # partitions (a, b, ci): [B, NB, 2, 2, 32, LH, LH] -> [B, NB, 128, LH, LH]
    xs = xs.reshape(NB, CIN_B, B, 2, 2, LH, LH).transpose(2, 0, 3, 4, 1, 5, 6)
    xt = np.ascontiguousarray(xs.reshape(B, NB, 128, LH, LH)).astype(bf)

    wt = np.zeros((NB, 2, 2, 128, 128), np.float32)
    for sh in range(2):
        for sw in range(2):
            for a in range(2):
                for bb in range(2):
                    for dh in range(2):
                        for dw in range(2):
                            kh = 2 * sh + a - dh
                            kw = 2 * sw + bb - dw
                            if 0 <= kh < KS and 0 <= kw < KS:
                                r0 = (a * 2 + bb) * CIN_B
                                c0 = (dh * 2 + dw) * COUT_B
                                wt[:, sh, sw, r0 : r0 + CIN_B,
                                   c0 : c0 + COUT_B] = weights_g[:, kh, kw]
    wt = wt.astype(bf)

    bg = np.broadcast_to(
        bias_bo.reshape(NB, 1, COUT_B), (NB, 4, COUT_B)
    ).reshape(NB, 128, 1).astype(np.float32)
    return xt, wt, np.ascontiguousarray(bg)


def _unpack_s2d2_output(yt_all):
    """yt_all: [NB, B, 128, 2, 512] -> [B, H, W, C] grouped fp32.

    partition = (dh, dw, co); col = (hc, ph<16, pw<32); h = 2*(16*hc+ph)+dh,
    w = 2*pw+dw.
    """
    yt_all = np.asarray(yt_all, dtype=np.float32)
    y = yt_all.reshape(NB, B, 2, 2, COUT_B, 2, 16, 32)
    # [B, hc, ph, dh, pw, dw, NB, co]
    y = y.transpose(1, 5, 6, 2, 7, 3, 0, 4)
    return np.ascontiguousarray(y.reshape(B, H, W, NB * COUT_B))


def _prep_s2d_inputs(xg, weights_g, bias_bo):
    """Host assembly for the s2d scheme.

    xg: [C, B, H, W] gathered grouped channels (fp32).
    weights_g: [NB, KS, KS, CIN_B, COUT_B] fp32.
    bias_bo: [C] bias in grouped-output order.
    Returns (xt, wt, bg) host arrays matching _build_nc_s2d's dram decls.
    """
    import ml_dtypes

    bf = ml_dtypes.bfloat16
    # Pad: h -> 66 (1 each side), w -> 68 = 4*17 (1 left, 3 right).
    xp = np.zeros((C, B, HP, DW * WCOL), np.float32)
    xp[:, :, 1 : H + 1, 1 : W + 1] = xg
    xs = xp.reshape(C, B, HP, WCOL, DW)  # [...,col,j]: w_pad = 4*col + j

    xt = np.empty((B, NB, CC, PW * CI_C, HP, WCOL), bf)
    for jr in range(PW):
        sub, sh = jr % DW, jr // DW
        # partition rows jr*CI_C..(jr+1)*CI_C: subgrid sub shifted sh cols
        src = np.zeros((C, B, HP, WCOL), np.float32)
        if sh == 0:
            src[:] = xs[:, :, :, :, sub]
        else:
            src[:, :, :, : WCOL - sh] = xs[:, :, :, sh:, sub]
        # src[c] for channel c: map to (g, cc, ci)
        s = src.reshape(NB, CC, CI_C, B, HP, WCOL)
        xt[:, :, :, jr * CI_C : (jr + 1) * CI_C] = s.transpose(3, 0, 1, 2, 4, 5)

    wt = np.zeros((NB, KS, CC, PW * CI_C, 128), np.float32)
    for jr in range(PW):
        for d in range(DW):
            kw = jr - d
            if 0 <= kw < KS:
                # [NB, KS(kh), CC*CI_C(ci), CO]
                wsrc = weights_g[:, :, kw].reshape(NB, KS, CC, CI_C, COUT_B)
                wt[:, :, :, jr * CI_C : (jr + 1) * CI_C,
                   d * COUT_B : (d + 1) * COUT_B] = wsrc
    wt = wt.astype(bf)

    # bias per partition (d, co) for group g: bias_bo[g*32+co], same all d.
    bg = np.broadcast_to(
        bias_bo.reshape(NB, 1, COUT_B), (NB, DW, COUT_B)
    ).reshape(NB, 128, 1).astype(np.float32)
    return xt, wt, np.ascontiguousarray(bg)


def _unpack_s2d_output(yt_all):
    """yt_all: [NB, B, 128, HCH, (H//HCH)*NPW] -> [B, H, W, C] grouped fp32."""
    yt_all = np.asarray(yt_all, dtype=np.float32)
    y = yt_all.reshape(NB, B, DW, COUT_B, HCH, H // HCH, NPW)
    # -> [B, h(=HCH*H//HCH), w(=NPW*DW), NB, COUT_B]
    y = y.transpose(1, 4, 5, 6, 2, 0, 3)  # [B, HCH, h', pw, d, NB, co]
    y = y.reshape(B, H, W, NB * COUT_B)
    return y


_BUILDERS = {"s2d2": _build_nc_s2d2, "s2d": _build_nc_s2d, "bd": _build_nc}


def _get_nc():
    if "nc" not in _NC_CACHE:
        _NC_CACHE["nc"] = _BUILDERS[SCHEME]()
    return _NC_CACHE["nc"]


def _build_timed(loop_k):
    return _BUILDERS[SCHEME](loop_k)


def _numpy_fallback(x, weights, bias, blocks_in, blocks_out):
    bi = blocks_in.reshape(-1)
    bo = blocks_out.reshape(-1)
    xg = x[..., bi]  # [B,H,W,NB*CIN_B]
    xp = np.zeros((B, HP, WP, NB * CIN_B), np.float32)
    xp[:, 1 : H + 1, 1 : W + 1] = xg
    y = np.zeros((B, H, W, NB * COUT_B), np.float32)
    wg = weights.astype(np.float32)
    for g in range(NB):
        acc = np.zeros((B, H, W, COUT_B), np.float32)
        for kh in range(KS):
            for kw in range(KS):
                patch = xp[:, kh : kh + H, kw : kw + W, g * CIN_B : (g + 1) * CIN_B]
                acc += patch @ wg[g, kh, kw]
        y[..., g * COUT_B : (g + 1) * COUT_B] = acc
    out = np.zeros((B, H, W, C), np.float32)
    np.add.at(out, (slice(None), slice(None), slice(None), bo), y)
    out += bias.astype(np.float32)
    return np.maximum(out, 0.0)


def kernel(x, weights, bias, blocks_in, blocks_out):
    x = np.asarray(x, dtype=np.float32)
    weights = np.asarray(weights, dtype=np.float32)
    bias = np.asarray(bias, dtype=np.float32)
    bi = np.asarray(blocks_in).reshape(-1)
    bo = np.asarray(blocks_out).reshape(-1)

    if np.unique(bo).size != NB * COUT_B:
        # Actual scatter collisions: rare/never per setup_inputs; keep correct.
        return _numpy_fallback(x, weights, bias, blocks_in, blocks_out)

    # Host-side gather (pure relabel) + pad + channel-major layout.
    xg = np.moveaxis(x[..., bi], -1, 0)  # [512, B, H, W], grouped channels

    global _LAST_IN_MAPS
    if SCHEME == "s2d2":
        xt, wt, bg = _prep_s2d2_inputs(xg, weights, bias[bo])
        in_maps = [
            {
                "xt": np.ascontiguousarray(xt[k * BPC : (k + 1) * BPC]),
                "wt": wt,
                "bg": bg,
            }
            for k in range(N_CORES)
        ]
        _LAST_IN_MAPS = in_maps
        nc = _get_nc()
        res = run_bass_kernel_spmd(nc, in_maps, list(range(N_CORES))).results
        yt_all = np.concatenate([res[k]["yt"] for k in range(N_CORES)], axis=1)
        yg = _unpack_s2d2_output(yt_all)
        out = np.empty((B, H, W, C), np.float32)
        out[..., bo] = yg
        return out

    if SCHEME == "s2d":
        xt, wt, bg = _prep_s2d_inputs(xg, weights, bias[bo])
        in_maps = [
            {
                "xt": np.ascontiguousarray(xt[k * BPC : (k + 1) * BPC]),
                "wt": wt,
                "bg": bg,
            }
            for k in range(N_CORES)
        ]
        _LAST_IN_MAPS = in_maps
        nc = _get_nc()
        res = run_bass_kernel_spmd(nc, in_maps, list(range(N_CORES))).results
        yt_all = np.concatenate(
            [res[k]["yt"] for k in range(N_CORES)], axis=1
        )  # [NB, B, 128, HCH, ...]
        yg = _unpack_s2d_output(yt_all)  # [B, H, W, C] grouped
        out = np.empty((B, H, W, C), np.float32)
        out[..., bo] = yg
        return out

    xt = np.zeros((C, B, HP, WP), np.float32)
    xt[:, :, 1 : H + 1, 1 : W + 1] = xg

    # Block-diagonal weight tiles [tap, ctile, 128, 128] (rows=cin, cols=cout).
    wt = np.zeros((KS * KS, NCT, 128, 128), np.float32)
    for g in range(NB):
        ct, j = divmod(g, GPT)
        wt[:, ct, j * CIN_B : (j + 1) * CIN_B, j * COUT_B : (j + 1) * COUT_B] = (
            weights[g].reshape(KS * KS, CIN_B, COUT_B)
        )

    bg = bias[bo].reshape(NCT, 128, 1).astype(np.float32)

    in_maps = []
    for k in range(N_CORES):
        shard = np.ascontiguousarray(xt[:, k * BPC : (k + 1) * BPC])
        in_maps.append({"xt": shard, "wt": wt, "bg": bg})

    _LAST_IN_MAPS = in_maps
    nc = _get_nc()
    res = run_bass_kernel_spmd(nc, in_maps, list(range(N_CORES))).results

    # [512, B, H, W] grouped-channel output -> scatter (relabel) to out.
    y = np.concatenate(
        [res[k]["yt"].reshape(C, BPC, H, W) for k in range(N_CORES)], axis=1
    )
    out = np.empty((B, H, W, C), np.float32)
    out[..., bo] = np.moveaxis(y, 0, -1)
    return out


# revision 31
# speedup vs baseline: 2.2572x; 1.0758x over previous
"""BlockConv2D Trainium2 kernel.

Reference computation (see harness): gather 16 blocks of 32 input channels
(indices blocks_in) from x[16,64,64,512], run a per-block 3x3 'same' conv
(weights [16,3,3,32,32]), scatter-add the 16x32 output channels back to 512
channels (indices blocks_out), add bias, relu.

Mapping: groups are independent 32->32 channel convs. Four groups' 32x32
weight blocks pack into one 128x128 block-diagonal stationary operand, so
each tap of the conv is a single matmul per 128-channel tile:
    psum[co_tile, spatial] += W[tap, ctile].T @ x[ctile, spatial+shift]
accumulated over the 9 taps in PSUM. Bias+ReLU fused on ScalarE.

Sharding: data-parallel over batch, 2 images per core across 8 cores.
The channel gather/scatter are permutations of 512 channels (disjoint
blocks), applied on host as pure relabeling; all arithmetic (conv, bias,
relu) runs on device. If blocks_out ever contains duplicates (scatter-add
semantics with actual collisions) we fall back to a numpy implementation.
"""

import numpy as np
from contextlib import ExitStack

import concourse.bass as bass
import concourse.tile as tile
from concourse import bacc, mybir
from concourse.bass_utils import run_bass_kernel_spmd

# Problem shape (hardcoded per contract).
B, H, W = 16, 64, 64
C = 512
NB, CIN_B, COUT_B = 16, 32, 32
KS = 3
N_CORES = 8
BPC = B // N_CORES          # images per core
HP, WP = H + 2, W + 2       # zero-padded input plane
SPAT_P = BPC * HP * WP      # padded spatial per core
SPAT_O = BPC * H * W        # output spatial per core
NCT = C // 128              # 128-channel tiles
GPT = 128 // CIN_B          # groups per channel tile

F32 = mybir.dt.float32
BF16 = mybir.dt.bfloat16
MM_DT = mybir.dt.float32r   # fp32 bits streamed in fast mode (1 cyc/row @ N>=256)

# 's2d': width space-to-depth scheme (37.5% PE util, bf16 inputs)
# 'bd': block-diagonal scheme (25% PE util, float32r)
SCHEME = "s2d2"

# s2d geometry: 4 output columns per stream column, 6 input positions,
# 16-channel ci chunks -> stationary [96, 128] per (group, kh, ci-chunk).
DW = 4                      # output cols packed per stream col
PW = 6                      # input w-positions in stationary rows
CC = 2                      # ci chunks of 16
CI_C = CIN_B // CC          # 16
WCOL = 17                   # w-s2d columns (padded W 68 = 4*17)
NPW = W // DW               # 16 output patches per row
HCH = 2                     # h chunks per image (32 rows x 16 patches = 512)

_NC_CACHE = {}


def _build_nc(loop_k=1):
    nc = bacc.Bacc(None, target_bir_lowering=False)
    xt_d = nc.dram_tensor("xt", [C, BPC, HP, WP], MM_DT, kind="ExternalInput")
    wt_d = nc.dram_tensor("wt", [KS * KS, NCT, 128, 128], MM_DT, kind="ExternalInput")
    bg_d = nc.dram_tensor("bg", [NCT, 128, 1], F32, kind="ExternalInput")
    yt_d = nc.dram_tensor("yt", [C, SPAT_O], F32, kind="ExternalOutput")

    with ExitStack() as ctx:
        tc = ctx.enter_context(tile.TileContext(nc))
        xpool = ctx.enter_context(tc.tile_pool(name="x", bufs=1))
        wpool = ctx.enter_context(tc.tile_pool(name="w", bufs=1))
        bpool = ctx.enter_context(tc.tile_pool(name="b", bufs=1))
        ypool = ctx.enter_context(tc.tile_pool(name="y", bufs=4))
        pspool = ctx.enter_context(
            tc.tile_pool(name="ps", bufs=8, space=bass.MemorySpace.PSUM)
        )

        # Weights: 9 taps x 4 ctiles of [128,128], one SBUF tile, per-tap DMAs
        # (so the first matmul only waits for its own tap, not the whole load).
        w_sb = wpool.tile([128, KS * KS * NCT * 128], MM_DT, tag="wsb")
        for t in range(KS * KS):
            nc.sync.dma_start(
                w_sb[:, t * NCT * 128 : (t + 1) * NCT * 128].rearrange(
                    "p (c m) -> p c m", c=NCT
                ),
                wt_d[t].rearrange("c p m -> p c m"),
            )

        b_sb = bpool.tile([128, NCT], F32, tag="bsb")
        nc.gpsimd.dma_start(b_sb[:], bg_d[:].rearrange("c p o -> p (c o)"))

        def body():
            # x resident in SBUF: one tile per (image, ctile): [128, 66, 66],
            # loaded as 3 row-band DMAs so compute starts after the first band.
            x_sb = {}
            for b in range(BPC):
                for ct in range(NCT):
                    xt_tile = xpool.tile([128, HP, WP], MM_DT, tag=f"x{b}_{ct}")
                    for r0, r1 in ((0, 22), (22, 44), (44, HP)):
                        nc.sync.dma_start(
                            xt_tile[:, r0:r1, :],
                            xt_d[ct * 128 : (ct + 1) * 128, b, r0:r1],
                        )
                    x_sb[(b, ct)] = xt_tile

            ROWS_PER_CHUNK = 8  # 8 output rows x 64 cols = 512 = one PSUM bank
            n_chunks = H // ROWS_PER_CHUNK
            for b in range(BPC):
                for oc in range(n_chunks):
                    r0 = oc * ROWS_PER_CHUNK
                    for ct in range(NCT):
                        ps = pspool.tile([128, ROWS_PER_CHUNK, W], F32)
                        for t in range(KS * KS):
                            kh, kw = divmod(t, KS)
                            rhs = x_sb[(b, ct)][
                                :, r0 + kh : r0 + kh + ROWS_PER_CHUNK, kw : kw + W
                            ]
                            nc.tensor.matmul(
                                ps[:],
                                w_sb[:, bass.ts(t * NCT + ct, 128)],
                                rhs,
                                start=(t == 0),
                                stop=(t == KS * KS - 1),
                            )
                        y_sb = ypool.tile([128, ROWS_PER_CHUNK * W], F32)
                        nc.scalar.activation(
                            y_sb[:],
                            ps[:].rearrange("p a b -> p (a b)"),
                            mybir.ActivationFunctionType.Relu,
                            bias=b_sb[:, ct : ct + 1],
                        )
                        off = b * H * W + r0 * W
                        nc.sync.dma_start(
                            yt_d[
                                ct * 128 : (ct + 1) * 128,
                                off : off + ROWS_PER_CHUNK * W,
                            ],
                            y_sb[:],
                        )

        if loop_k == 1:
            body()
        else:
            with tc.For_i(0, loop_k, 1):
                body()
    nc.compile()
    return nc


def _build_nc_s2d(loop_k=1):
    """Width space-to-depth grouped conv.

    Stream column = (h, pw): 4 output pixels w=4*pw+d, d<4, of one group.
    Stationary [96=(jr<6, ci<16), 128=(d<4, co<32)] holds w[g,kh,jr-d,ci,co]
    (kw = jr-d in 0..2). rhs rows jr carry x at padded w = 4*pw+jr, i.e.
    w-subgrid jr%4 shifted by jr//4 columns — host pre-assembles the 6-row
    tiles (1.5x input replication, bf16). kh accumulates in PSUM (3 passes
    x 2 ci chunks = 6 matmuls per 512-col PSUM bank).
    """
    nc = bacc.Bacc(None, target_bir_lowering=False, num_swdge_queues=4)
    xt_d = nc.dram_tensor(
        "xt", [BPC, NB, CC, PW * CI_C, HP, WCOL], BF16, kind="ExternalInput"
    )
    wt_d = nc.dram_tensor(
        "wt", [NB, KS, CC, PW * CI_C, 128], BF16, kind="ExternalInput"
    )
    bg_d = nc.dram_tensor("bg", [NB, 128, 1], F32, kind="ExternalInput")
    yt_d = nc.dram_tensor(
        "yt", [NB, BPC, 128, HCH, H // HCH * NPW], BF16, kind="ExternalOutput"
    )

    with ExitStack() as ctx:
        tc = ctx.enter_context(tile.TileContext(nc))
        xpool = ctx.enter_context(tc.tile_pool(name="x", bufs=10))
        wpool = ctx.enter_context(tc.tile_pool(name="w", bufs=1))
        bpool = ctx.enter_context(tc.tile_pool(name="b", bufs=1))
        ypool = ctx.enter_context(tc.tile_pool(name="y", bufs=6))
        pspool = ctx.enter_context(
            tc.tile_pool(name="ps", bufs=8, space=bass.MemorySpace.PSUM)
        )

        # Stationaries resident: [96, NB*KS*CC*128] bf16; per-group DMAs
        # are issued lazily inside the body so group 0 compute starts early.
        wn = KS * CC * 128
        w_sb = wpool.tile([PW * CI_C, NB * wn], BF16, tag="wsb")
        w_loaded = set()

        def load_w(g):
            if g in w_loaded:
                return
            w_loaded.add(g)
            nc.sync.dma_start(
                w_sb[:, g * wn : (g + 1) * wn].rearrange(
                    "p (a c m) -> p a c m", a=KS, c=CC
                ),
                wt_d[g].rearrange("a c p m -> p a c m"),
            )

        b_sb = bpool.tile([128, NB], F32, tag="bsb")
        nc.sync.dma_start(b_sb[:], bg_d[:].rearrange("g p o -> p (g o)"))

        def body():
            for b in range(BPC):
                for g in range(NB):
                    load_w(g)
                    load_w(min(g + 1, NB - 1))
                    load_w(min(g + 2, NB - 1))
                    xt = {}
                    for cc in range(CC):
                        t = xpool.tile([PW * CI_C, HP, WCOL], BF16, tag=f"xc{cc}")
                        nc.sync.dma_start(t[:], xt_d[b, g, cc])
                        xt[cc] = t
                    for hc in range(HCH):
                        h0 = hc * (H // HCH)
                        ps = pspool.tile([128, H // HCH, NPW], F32)
                        first, last = (0, 0), (KS - 1, CC - 1)
                        for kh in range(KS):
                            for cc in range(CC):
                                rhs = xt[cc][
                                    :, h0 + kh : h0 + kh + H // HCH, 0:NPW
                                ]
                                off = (g * KS + kh) * CC + cc
                                nc.tensor.matmul(
                                    ps[:],
                                    w_sb[:, bass.ts(off, 128)],
                                    rhs,
                                    start=((kh, cc) == first),
                                    stop=((kh, cc) == last),
                                )
                        y_sb = ypool.tile([128, H // HCH * NPW], BF16)
                        nc.scalar.activation(
                            y_sb[:],
                            ps[:].rearrange("p a b -> p (a b)"),
                            mybir.ActivationFunctionType.Relu,
                            bias=b_sb[:, g : g + 1],
                        )
                        nc.gpsimd.dma_start(yt_d[g, b, :, hc], y_sb[:])

        if loop_k == 1:
            body()
        else:
            with tc.For_i(0, loop_k, 1, hint_engines=(mybir.EngineType.PE,)):
                body()
    nc.compile()
    return nc


def _build_nc_s2d2(loop_k=1):
    """2x2-patch space-to-depth grouped conv, 56.25% PE utilization.

    x is decomposed 2x2 (padded coords): subgrid (a,b)[lh,lw] = xpad[2lh+a,
    2lw+b]. A stream column is one 2x2 output patch (ph,pw); the four input
    positions it needs in each axis fold into parity a/b (partition dim) and
    shift s in {0,1} (a pure AP offset on the same tile). Four shift-matmuls
    (s_h,s_w), each [128=(a,b,ci=32), 128=(dh,dw,co)], cover all 9 taps:
    stationary cell ((a,b,ci),(dh,dw,co)) = w[g, 2*s_h+a-dh, 2*s_w+b-dw, ci,
    co] (zero outside 0..2). No input replication, full ci per pass.
    """
    nc = bacc.Bacc(None, target_bir_lowering=False, num_swdge_queues=4)
    LH = HP // 2  # 33 subgrid lines per axis
    xt_d = nc.dram_tensor("xt", [BPC, NB, 128, LH, LH], BF16, kind="ExternalInput")
    wt_d = nc.dram_tensor("wt", [NB, 2, 2, 128, 128], BF16, kind="ExternalInput")
    bg_d = nc.dram_tensor("bg", [NB, 128, 1], F32, kind="ExternalInput")
    # psum chunk = (ph 16, pw 32) = 512 cols; 2 chunks per image.
    yt_d = nc.dram_tensor(
        "yt", [NB, BPC, 128, 2, 512], BF16, kind="ExternalOutput"
    )

    with ExitStack() as ctx:
        tc = ctx.enter_context(tile.TileContext(nc))
        xpool = ctx.enter_context(tc.tile_pool(name="x", bufs=6))
        wpool = ctx.enter_context(tc.tile_pool(name="w", bufs=1))
        bpool = ctx.enter_context(tc.tile_pool(name="b", bufs=1))
        ypool = ctx.enter_context(tc.tile_pool(name="y", bufs=6))
        pspool = ctx.enter_context(
            tc.tile_pool(name="ps", bufs=8, space=bass.MemorySpace.PSUM)
        )

        w_sb = wpool.tile([128, NB * 4 * 128], BF16, tag="wsb")
        w_loaded = set()

        def load_w(g):
            if g in w_loaded:
                return
            w_loaded.add(g)
            nc.sync.dma_start(
                w_sb[:, g * 512 : (g + 1) * 512].rearrange(
                    "p (a b m) -> p a b m", a=2, b=2
                ),
                wt_d[g].rearrange("a b p m -> p a b m"),
            )

        b_sb = bpool.tile([128, NB], F32, tag="bsb")
        nc.sync.dma_start(b_sb[:], bg_d[:].rearrange("g p o -> p (g o)"))

        def body():
            for b in range(BPC):
                for g in range(NB):
                    load_w(g)
                    load_w(min(g + 1, NB - 1))
                    load_w(min(g + 2, NB - 1))
                    xt = xpool.tile([128, LH, LH], BF16, tag="xtile")
                    nc.sync.dma_start(xt[:], xt_d[b, g])
                    y_sb = ypool.tile([128, 2, 512], BF16)
                    for hc in range(2):
                        ps = pspool.tile([128, 16, 32], F32)
                        for sh in range(2):
                            for sw in range(2):
                                rhs = xt[
                                    :,
                                    hc * 16 + sh : hc * 16 + sh + 16,
                                    sw : sw + 32,
                                ]
                                nc.tensor.matmul(
                                    ps[:],
                                    w_sb[:, bass.ts(g * 4 + sh * 2 + sw, 128)],
                                    rhs,
                                    start=(sh == 0 and sw == 0),
                                    stop=(sh == 1 and sw == 1),
                                )
                        nc.scalar.activation(
                            y_sb[:, hc],
                            ps[:].rearrange("p a b -> p (a b)"),
                            mybir.ActivationFunctionType.Relu,
                            bias=b_sb[:, g : g + 1],
                        )
                    nc.gpsimd.dma_start(yt_d[g, b], y_sb[:])

        if loop_k == 1:
            body()
        else:
            with tc.For_i(0, loop_k, 1, hint_engines=(mybir.EngineType.PE,)):
                body()
    nc.compile()
    return nc


def _prep_s2d2_inputs(xg, weights_g, bias_bo):
    """Host assembly for the 2x2-patch s2d scheme.

    xg: [C, B, H, W] gathered grouped channels (fp32).
    Returns (xt, wt, bg) matching _build_nc_s2d2's dram decls.
    """
    import ml_dtypes

    bf = ml_dtypes.bfloat16
    LH = HP // 2
    xp = np.zeros((C, B, HP, HP), np.float32)
    xp[:, :, 1 : H + 1, 1 : W + 1] = xg
    # subgrid (a,b): xp[2lh+a, 2lw+b] -> [C, B, a, b, LH, LH]
    xs = xp.reshape(C, B, LH, 2, LH, 2).transpose(0, 1, 3, 5, 2, 4)
    # partitions (a, b, ci): [B, NB, 2, 2, 32, LH, LH] -> [B, NB, 128, LH, LH]
    xs = xs.reshape(NB, CIN_B, B, 2, 2, LH, LH).transpose(2, 0, 3, 4, 1, 5, 6)
    xt = np.ascontiguousarray(xs.reshape(B, NB, 128, LH, LH)).astype(bf)

    wt = np.zeros((NB, 2, 2, 128, 128), np.float32)
    for sh in range(2):
        for sw in range(2):
            for a in range(2):
                for bb in range(2):
                    for dh in range(2):
                        for dw in range(2):
                            kh = 2 * sh + a - dh
                            kw = 2 * sw + bb - dw
                            if 0 <= kh < KS and 0 <= kw < KS:
                                r0 = (a * 2 + bb) * CIN_B
                                c0 = (dh * 2 + dw) * COUT_B
                                wt[:, sh, sw, r0 : r0 + CIN_B,
                                   c0 : c0 + COUT_B] = weights_g[:, kh, kw]
    wt = wt.astype(bf)

    bg = np.broadcast_to(
        bias_bo.reshape(NB, 1, COUT_B), (NB, 4, COUT_B)
    ).reshape(NB, 128, 1).astype(np.float32)
    return xt, wt, np.ascontiguousarray(bg)


def _unpack_s2d2_output(yt_all):
    """yt_all: [NB, B, 128, 2, 512] -> [B, H, W, C] grouped fp32.

    partition = (dh, dw, co); col = (hc, ph<16, pw<32); h = 2*(16*hc+ph)+dh,
    w = 2*pw+dw.
    """
    yt_all = np.asarray(yt_all, dtype=np.float32)
    y = yt_all.reshape(NB, B, 2, 2, COUT_B, 2, 16, 32)
    # [B, hc, ph, dh, pw, dw, NB, co]
    y = y.transpose(1, 5, 6, 2, 7, 3, 0, 4)
    return np.ascontiguousarray(y.reshape(B, H, W, NB * COUT_B))


def _prep_s2d_inputs(xg, weights_g, bias_bo):
    """Host assembly for the s2d scheme.

    xg: [C, B, H, W] gathered grouped channels (fp32).
    weights_g: [NB, KS, KS, CIN_B, COUT_B] fp32.
    bias_bo: [C] bias in grouped-output order.
    Returns (xt, wt, bg) host arrays matching _build_nc_s2d's dram decls.
    """
    import ml_dtypes

    bf = ml_dtypes.bfloat16
    # Pad: h -> 66 (1 each side), w -> 68 = 4*17 (1 left, 3 right).
    xp = np.zeros((C, B, HP, DW * WCOL), np.float32)
    xp[:, :, 1 : H + 1, 1 : W + 1] = xg
    xs = xp.reshape(C, B, HP, WCOL, DW)  # [...,col,j]: w_pad = 4*col + j

    xt = np.empty((B, NB, CC, PW * CI_C, HP, WCOL), bf)
    for jr in range(PW):
        sub, sh = jr % DW, jr // DW
        # partition rows jr*CI_C..(jr+1)*CI_C: subgrid sub shifted sh cols
        src = np.zeros((C, B, HP, WCOL), np.float32)
        if sh == 0:
            src[:] = xs[:, :, :, :, sub]
        else:
            src[:, :, :, : WCOL - sh] = xs[:, :, :, sh:, sub]
        # src[c] for channel c: map to (g, cc, ci)
        s = src.reshape(NB, CC, CI_C, B, HP, WCOL)
        xt[:, :, :, jr * CI_C : (jr + 1) * CI_C] = s.transpose(3, 0, 1, 2, 4, 5)

    wt = np.zeros((NB, KS, CC, PW * CI_C, 128), np.float32)
    for jr in range(PW):
        for d in range(DW):
            kw = jr - d
            if 0 <= kw < KS:
                # [NB, KS(kh), CC*CI_C(ci), CO]
                wsrc = weights_g[:, :, kw].reshape(NB, KS, CC, CI_C, COUT_B)
                wt[:, :, :, jr * CI_C : (jr + 1) * CI_C,
                   d * COUT_B : (d + 1) * COUT_B] = wsrc
    wt = wt.astype(bf)

    # bias per partition (d, co) for group g: bias_bo[g*32+co], same all d.
    bg = np.broadcast_to(
        bias_bo.reshape(NB, 1, COUT_B), (NB, DW, COUT_B)
    ).reshape(NB, 128, 1).astype(np.float32)
    return xt, wt, np.ascontiguousarray(bg)


def _unpack_s2d_output(yt_all):
    """yt_all: [NB, B, 128, HCH, (H//HCH)*NPW] -> [B, H, W, C] grouped fp32."""
    yt_all = np.asarray(yt_all, dtype=np.float32)
    y = yt_all.reshape(NB, B, DW, COUT_B, HCH, H // HCH, NPW)
    # -> [B, h(=HCH*H//HCH), w(=NPW*DW), NB, COUT_B]
    y = y.transpose(1, 4, 5, 6, 2, 0, 3)  # [B, HCH, h', pw, d, NB, co]
    y = y.reshape(B, H, W, NB * COUT_B)
    return y


_BUILDERS = {"s2d2": _build_nc_s2d2, "s2d": _build_nc_s2d, "bd": _build_nc}


def _get_nc():
    if "nc" not in _NC_CACHE:
        _NC_CACHE["nc"] = _BUILDERS[SCHEME]()
    return _NC_CACHE["nc"]


def _build_timed(loop_k):
    return _BUILDERS[SCHEME](loop_k)


def _numpy_fallback(x, weights, bias, blocks_in, blocks_out):
    bi = blocks_in.reshape(-1)
    bo = blocks_out.reshape(-1)
    xg = x[..., bi]  # [B,H,W,NB*CIN_B]
    xp = np.zeros((B, HP, WP, NB * CIN_B), np.float32)
    xp[:, 1 : H + 1, 1 : W + 1] = xg
    y = np.zeros((B, H, W, NB * COUT_B), np.float32)
    wg = weights.astype(np.float32)
    for g in range(NB):
        acc = np.zeros((B, H, W, COUT_B), np.float32)
        for kh in range(KS):
            for kw in range(KS):
                patch = xp[:, kh : kh + H, kw : kw + W, g * CIN_B : (g + 1) * CIN_B]
                acc += patch @ wg[g, kh, kw]
        y[..., g * COUT_B : (g + 1) * COUT_B] = acc
    out = np.zeros((B, H, W, C), np.float32)
    np.add.at(out, (slice(None), slice(None), slice(None), bo), y)
    out += bias.astype(np.float32)
    return np.maximum(out, 0.0)


def kernel(x, weights, bias, blocks_in, blocks_out):
    x = np.asarray(x, dtype=np.float32)
    weights = np.asarray(weights, dtype=np.float32)
    bias = np.asarray(bias, dtype=np.float32)
    bi = np.asarray(blocks_in).reshape(-1)
    bo = np.asarray(blocks_out).reshape(-1)

    if np.unique(bo).size != NB * COUT_B:
        # Actual scatter collisions: rare/never per setup_inputs; keep correct.
        return _numpy_fallback(x, weights, bias, blocks_in, blocks_out)

    # Host-side gather (pure relabel) + pad + channel-major layout.
    xg = np.moveaxis(x[..., bi], -1, 0)  # [512, B, H, W], grouped channels

    global _LAST_IN_MAPS
    if SCHEME == "s2d2":
        xt, wt, bg = _prep_s2d2_inputs(xg, weights, bias[bo])
        in_maps = [
            {
                "xt": np.ascontiguousarray(xt[k * BPC : (k + 1) * BPC]),
                "wt": wt,
                "bg": bg,
            }
            for k in range(N_CORES)
        ]
        _LAST_IN_MAPS = in_maps
        nc = _get_nc()
        res = run_bass_kernel_spmd(nc, in_maps, list(range(N_CORES))).results
        yt_all = np.concatenate([res[k]["yt"] for k in range(N_CORES)], axis=1)
        yg = _unpack_s2d2_output(yt_all)
        out = np.empty((B, H, W, C), np.float32)
        out[..., bo] = yg
        return out

    if SCHEME == "s2d":
        xt, wt, bg = _prep_s2d_inputs(xg, weights, bias[bo])
        in_maps = [
            {
                "xt": np.ascontiguousarray(xt[k * BPC : (k + 1) * BPC]),
                "wt": wt,
                "bg": bg,
            }
            for k in range(N_CORES)
        ]
        _LAST_IN_MAPS = in_maps
        nc = _get_nc()
        res = run_bass_kernel_spmd(nc, in_maps, list(range(N_CORES))).results
        yt_all = np.concatenate(
            [res[k]["yt"] for k in range(N_CORES)], axis=1
        )  # [NB, B, 128, HCH, ...]
        yg = _unpack_s2d_output(yt_all)  # [B, H, W, C] grouped
        out = np.empty((B, H, W, C), np.float32)
        out[..., bo] = yg
        return out

    xt = np.zeros((C, B, HP, WP), np.float32)
    xt[:, :, 1 : H + 1, 1 : W + 1] = xg

    # Block-diagonal weight tiles [tap, ctile, 128, 128] (rows=cin, cols=cout).
    wt = np.zeros((KS * KS, NCT, 128, 128), np.float32)
    for g in range(NB):
        ct, j = divmod(g, GPT)
        wt[:, ct, j * CIN_B : (j + 1) * CIN_B, j * COUT_B : (j + 1) * COUT_B] = (
            weights[g].reshape(KS * KS, CIN_B, COUT_B)
        )

    bg = bias[bo].reshape(NCT, 128, 1).astype(np.float32)

    in_maps = []
    for k in range(N_CORES):
        shard = np.ascontiguousarray(xt[:, k * BPC : (k + 1) * BPC])
        in_maps.append({"xt": shard, "wt": wt, "bg": bg})

    _LAST_IN_MAPS = in_maps
    nc = _get_nc()
    res = run_bass_kernel_spmd(nc, in_maps, list(range(N_CORES))).results

    # [512, B, H, W] grouped-channel output -> scatter (relabel) to out.
    y = np.concatenate(
        [res[k]["yt"].reshape(C, BPC, H, W) for k in range(N_CORES)], axis=1
    )
    out = np.empty((B, H, W, C), np.float32)
    out[..., bo] = np.moveaxis(y, 0, -1)
    return out
